# revision 12
# baseline (speedup 1.0000x reference)
"""CrossVariableAttention Bass/Tile kernel for TRN2.

Per-core program (data parallel over batch, one batch element per core).
Host-side algebraic fusions (see _make_in_maps):

  scores:  S^T[m,n] = X[:,m]^T.B^T.X[:,n] + r[m],  B := Wq'.Wk^T, r = (Wk.bq')^T.X
  output:  VP := X^T.(Wv.Wp) + bv.Wp;  y_un^T = VP^T.Pt;  y = y_un*recip + bp

Schedule (all matmuls f32r, PE kept dense):

  head:    DMA issues split across engine SWDGE queues in need-order, since
           concurrent transfers share HBM bandwidth: sync carries wb per-IC
           tiles (first matmul group needs just wb_ic0 + x slab0 ic0), gpsimd
           carries the 16 per-ic x tiles, scalar carries u then the late
           consts.  C is ic-outer everywhere so compute chases arrivals.
  phase 1: C[d,n] = WB^T.X and r = u^T.X (ic-outer, 5 live psum tiles),
           VP = X^T.Wvp + bvp.
  phase 2, per slab of 512 queries (PE order, no idle):
    S^T = X^T.C        [64 mm]   exp on ACT (bias r[m]), Pt-sum chain on DVE
    Y^T oc0            [16 mm]
    bcden = ones128^T.acc [1 mm] -> psum [128,512] = den broadcast to all
           partitions (replaces den-mm + DRAM-bounce broadcast)
    DVE reciprocal (psum -> sbuf) runs during Y oc1..3
    Y^T oc1..3         [48 mm]
    per oc: DVE mult+bias, store; the very last tile is processed in two
    column halves down two DMA engines to shorten the tail.
"""

from contextlib import ExitStack

import concourse.bass as bass
import concourse.mybir as mybir
import concourse.tile as tile
from concourse.bass import ds
from concourse.vector_clock import ScopedClock

F32 = mybir.dt.float32
F32R = mybir.dt.float32r
F16 = mybir.dt.float16
AF = mybir.ActivationFunctionType

P = 128
D = 512
N = 2048
DCH = D // P         # 4 d chunks
NCH = N // P         # 16 token chunks (m)
NSLAB = N // 512     # 4 query slabs


# ---------------------------------------------------------------------------
# The walrus build in this env accepts at most ONE sync wait per instruction
# (setupSyncWait: "Too many sync wait commands").  Tile attaches several.
# Fix: split excess waits onto engine-local NOPs placed just before the
# instruction (same engine => same stream order => identical semantics).
MAX_WAITS_PER_INST = 1


class SplitDrainTileContext(tile.TileContext):
    def _drain_and_barrier(self, tick_clock, wait_clock):
        nc = self.nc
        probe = nc.sync.nop(nofuse=True, hint="split_drain_waits")
        wait_clock.add_sem_waits(
            probe.ins, ScopedClock({None: tick_clock.global_clock})
        )
        waits = list(probe.ins.sync_info.on_wait)
        probe.ins.sync_info.on_wait = waits[:MAX_WAITS_PER_INST]
        for i in range(MAX_WAITS_PER_INST, len(waits), MAX_WAITS_PER_INST):
            extra = nc.sync.nop(nofuse=True, hint="split_drain_waits")
            extra.ins.sync_info = mybir.SyncInfo(
                on_wait=waits[i : i + MAX_WAITS_PER_INST], on_update=[]
            )
        nc.sync.drain()
        nc.all_engine_barrier()
        assert self.sems is not None
        popped = nc._tile_sem_poison_stack.pop()
        assert popped is self._sem_poison
        nc.clear_and_free_semaphores(list(self.sems.allocated().values()))
        nc.all_engine_barrier()


def split_sync_waits(nc, max_waits=MAX_WAITS_PER_INST):
    n_split = 0
    for fn in nc.m.functions:
        for bb in fn.blocks:
            insts = list(bb.instructions)
            out = []
            changed = False
            for inst in insts:
                si = getattr(inst, "sync_info", None)
                if si is not None:
                    waits = list(si.on_wait or [])
                    if len(waits) > max_waits:
                        changed = True
                        for j, w in enumerate(waits[: len(waits) - max_waits]):
                            out.append(
                                mybir.InstNoOp(
                                    name=f"{inst.name}-sw{j}",
                                    engine=inst.engine,
                                    bass_nofuse=True,
                                    sync_info=mybir.SyncInfo(
                                        on_wait=[w], on_update=[]
                                    ),
                                )
                            )
                            n_split += 1
                        si.on_wait = waits[len(waits) - max_waits :]
                out.append(inst)
            if changed:
                bb.instructions = out
    return n_split


def build_nc():
    nc = bass.Bass()

    x = nc.declare_dram_parameter("x", [D, N], F16, isOutput=False)
    # wb[ic][p, oc, col] = B[ic*128+p, oc*128+col]
    wb = nc.declare_dram_parameter("wb", [DCH, P, DCH, P], F16, isOutput=False)
    wvp = nc.declare_dram_parameter("wvp", [D, D], F16, isOutput=False)
    u = nc.declare_dram_parameter("u", [P, DCH], F16, isOutput=False)
    ones_in = nc.declare_dram_parameter("ones", [P, P], F16, isOutput=False)
    bvp = nc.declare_dram_parameter("bvp", [D], F32, isOutput=False)
    y = nc.declare_dram_parameter("y", [D, N], F32, isOutput=True)
    r_dram = nc.dram_tensor("r_scratch", [N], F32)

    with SplitDrainTileContext(nc) as tc, ExitStack() as ctx:
        consts = ctx.enter_context(tc.tile_pool(name="consts", bufs=1))
        big = ctx.enter_context(tc.tile_pool(name="big", bufs=1))
        small = ctx.enter_context(tc.tile_pool(name="small", bufs=2))

        u_sb = consts.tile([P, DCH], F16, tag="u")
        ones_sq = consts.tile([P, P], F16, tag="ones")
        bvp_bc = consts.tile([P, D], F32, tag="bvp")
        wvp_sb = consts.tile([P, DCH, D], F16, tag="wvp")
        rcol_sb = consts.tile([P, NCH], F32, tag="rcol")
        wb_tiles = [
            consts.tile([P, DCH, P], F16, tag=f"wb{ic}", name=f"wb{ic}")
            for ic in range(DCH)
        ]

        # --- persistent big tensors --------------------------------------
        c_sb = big.tile([P, DCH, N], F16, tag="c")
        vp_sb = big.tile([P, NCH, D], F16, tag="vp")
        # x arrives as 16 per-(slab, ic) [P, 512] tiles so compute can chase
        x_ic = [
            [
                big.tile([P, 512], F16, tag=f"x{nb}i{ic}", name=f"x{nb}i{ic}")
                for ic in range(DCH)
            ]
            for nb in range(NSLAB)
        ]

        def xl(mc, ic):
            """lhsT chunk [128, 128] of X^T for token chunk mc, d chunk ic."""
            return x_ic[mc // 4][ic][:, ds((mc % 4) * P, P)]

        x_re = x.rearrange("(c p) n -> p c n", p=P)

        # --- PE warm-up: dummy matmuls on garbage SBUF keep the HAM clock
        # gate busy while the first input DMAs are in flight (results unused)
        with tc.tile_pool(name="ps_w", bufs=1, space="PSUM") as ps_w:
            warm_ps = ps_w.tile([P, 512], F32, tag="warm")
            for _ in range(24):
                nc.tensor.matmul(
                    warm_ps[:, ds(0, 256)], c_sb[:, 0, ds(0, P)],
                    c_sb[:, 0, ds(0, 256)], start=True, stop=True,
                )

        # --- DMA issue plan: need-order; concurrent transfers share HBM BW
        nc.sync.dma_start(out=wb_tiles[0], in_=wb[0])
        nc.gpsimd.dma_start(out=x_ic[0][0], in_=x_re[:, 0, ds(0, 512)])
        nc.scalar.dma_start(out=u_sb, in_=u[:, :])
        for ic in range(1, DCH):
            nc.sync.dma_start(out=wb_tiles[ic], in_=wb[ic])
        # x slabs 0-1: issues split across gpsimd+scalar to halve the
        # issue-serialization on the tiles phase 1 chases
        for nb in range(2):
            for ic in range(DCH):
                if nb == 0 and ic == 0:
                    continue
                eng = nc.gpsimd if ic % 2 == 0 else nc.scalar
                eng.dma_start(
                    out=x_ic[nb][ic], in_=x_re[:, ic, ds(nb * 512, 512)]
                )
        nc.gpsimd.dma_start(out=wvp_sb, in_=wvp.rearrange("(c p) o -> p c o", p=P))
        for nb in range(2, NSLAB):
            for ic in range(DCH):
                nc.gpsimd.dma_start(
                    out=x_ic[nb][ic], in_=x_re[:, ic, ds(nb * 512, 512)]
                )

        # --- phase 1: C = WB^T.X, r = u^T.X, VP = X^T.Wvp + bvp ----------
        with tc.tile_pool(name="ps1", bufs=7, space="PSUM") as ps1, \
             tc.tile_pool(name="ps_r", bufs=1, space="PSUM") as ps_r:
            for nb in range(NSLAB):
                psC = [
                    ps1.tile([P, 512], F32, tag="ps1", name=f"psC{nb}_{oc}")
                    for oc in range(DCH)
                ]
                psr = ps_r.tile([1, 512], F32, tag="psr")
                for ic in range(DCH):
                    for oc in range(DCH):
                        nc.tensor.matmul(
                            psC[oc],
                            wb_tiles[ic][:, oc, :],
                            x_ic[nb][ic],
                            start=(ic == 0),
                            stop=(ic == DCH - 1),
                        )
                    nc.tensor.matmul(
                        psr,
                        u_sb[:, ic : ic + 1],
                        x_ic[nb][ic],
                        start=(ic == 0),
                        stop=(ic == DCH - 1),
                    )
                for oc in range(DCH):
                    nc.scalar.copy(out=c_sb[:, oc, ds(nb * 512, 512)], in_=psC[oc])
                r_sb = small.tile([1, 512], F32, tag="rsb")
                nc.vector.tensor_copy(out=r_sb, in_=psr)
                nc.sync.dma_start(out=r_dram[ds(nb * 512, 512)], in_=r_sb)
                if nb == 0:
                    # late consts: issue once the head-critical window passed
                    nc.scalar.dma_start(out=ones_sq, in_=ones_in[:, :])
                    bvp_ap = bvp[:]
                    nc.scalar.dma_start(
                        out=bvp_bc,
                        in_=bass.AP(
                            tensor=bvp_ap.tensor, offset=bvp_ap.offset,
                            ap=[[0, P], bvp_ap.ap[0]],
                        ),
                    )

            # VP = X^T . Wvp + bvp
            for mc in range(NCH):
                ps = ps1.tile([P, 512], F32, tag="ps1", name=f"psV{mc}")
                for ic in range(DCH):
                    nc.tensor.matmul(
                        ps,
                        xl(mc, ic),
                        wvp_sb[:, ic, :],
                        start=(ic == 0),
                        stop=(ic == DCH - 1),
                    )
                nc.vector.tensor_add(out=vp_sb[:, mc, :], in0=ps, in1=bvp_bc)

            # r in column layout [128, 16]: rcol[p, mc] = r[mc*128 + p]
            nc.sync.dma_start(
                out=rcol_sb, in_=r_dram.rearrange("(c p) -> p c", p=P)
            )

        # --- phase 2: attention, per slab of 512 queries ------------------
        with tc.tile_pool(name="pt", bufs=20) as pt_pool, \
             tc.tile_pool(name="outp", bufs=4) as outp, \
             tc.tile_pool(name="ps_st", bufs=4, space="PSUM") as ps_st, \
             tc.tile_pool(name="ps_y", bufs=3, space="PSUM") as ps_y, \
             tc.tile_pool(name="ps_bc", bufs=1, space="PSUM") as ps_bc:
            for nb in range(NSLAB):
                nsl = ds(nb * 512, 512)

                # S^T tiles + exp(S + r); DVE accumulates Pt running sum so
                # the denominator needs only ONE matmul (the broadcast one)
                pt_tiles = []
                acc = small.tile([P, 512], F16, tag="denacc")
                for mc in range(NCH):
                    ps = ps_st.tile([P, 512], F32, tag="st")
                    for ic in range(DCH):
                        nc.tensor.matmul(
                            ps,
                            xl(mc, ic),
                            c_sb[:, ic, nsl],
                            start=(ic == 0),
                            stop=(ic == DCH - 1),
                        )
                    pt = pt_pool.tile([P, 512], F16, tag="pt")
                    nc.scalar.activation(
                        out=pt,
                        in_=ps,
                        func=AF.Exp,
                        bias=rcol_sb[:, mc : mc + 1],
                        scale=1.0,
                    )
                    pt_tiles.append(pt)
                    if mc == 1:
                        nc.vector.tensor_add(
                            out=acc, in0=pt_tiles[0], in1=pt_tiles[1]
                        )
                    elif mc > 1:
                        nc.vector.tensor_add(out=acc, in0=acc, in1=pt)

                # Y^T oc0 first: gives the DVE acc chain time to finish
                ps_ys = []
                ps0y = ps_y.tile([P, 512], F32, tag="y")
                ps_ys.append(ps0y)
                for mc in range(NCH):
                    nc.tensor.matmul(
                        ps0y,
                        vp_sb[:, mc, ds(0, P)],
                        pt_tiles[mc][:, :],
                        start=(mc == 0),
                        stop=(mc == NCH - 1),
                    )

                # denominator broadcast: ones128^T @ acc -> [128, 512] psum
                ps_d = ps_bc.tile([P, 512], F32, tag="bcden")
                nc.tensor.matmul(
                    ps_d, ones_sq[:, :], acc[:, :], start=True, stop=True
                )
                recip_bc = small.tile([P, 512], F32, tag="recip_bc")
                nc.vector.reciprocal(out=recip_bc, in_=ps_d)

                # Y^T oc1..3 (last slab: oc3 in two half-width groups so
                # the first half's store overlaps the second half's matmuls)
                last_slab = nb == NSLAB - 1
                for oc in range(1, DCH):
                    if last_slab and oc == DCH - 1:
                        break
                    ps = ps_y.tile([P, 512], F32, tag="y")
                    ps_ys.append(ps)
                    for mc in range(NCH):
                        nc.tensor.matmul(
                            ps,
                            vp_sb[:, mc, ds(oc * P, P)],
                            pt_tiles[mc][:, :],
                            start=(mc == 0),
                            stop=(mc == NCH - 1),
                        )

                # normalize + store (per oc, overlapped with Y)
                for oc in range(DCH - 1 if last_slab else DCH):
                    t = outp.tile([P, 512], F32, tag="out")
                    nc.vector.tensor_tensor(
                        out=t, in0=ps_ys[oc], in1=recip_bc,
                        op=mybir.AluOpType.mult,
                    )
                    nc.sync.dma_start(out=y[ds(oc * P, P), nsl], in_=t)

                if last_slab:
                    oc = DCH - 1
                    for h, eng in ((0, nc.sync), (1, nc.scalar)):
                        psh = ps_y.tile([P, 256], F32, tag="y", name=f"psh{h}")
                        for mc in range(NCH):
                            nc.tensor.matmul(
                                psh,
                                vp_sb[:, mc, ds(oc * P, P)],
                                pt_tiles[mc][:, ds(h * 256, 256)],
                                start=(mc == 0),
                                stop=(mc == NCH - 1),
                            )
                        t = outp.tile(
                            [P, 256], F32, tag=f"outh{h}", name=f"outh{h}"
                        )
                        nc.vector.tensor_tensor(
                            out=t,
                            in0=psh,
                            in1=recip_bc[:, ds(h * 256, 256)],
                            op=mybir.AluOpType.mult,
                        )
                        eng.dma_start(
                            out=y[ds(oc * P, P), ds(nb * 512 + h * 256, 256)],
                            in_=t,
                        )

    split_sync_waits(nc)
    return nc


import numpy as np
from concourse.bass_utils import run_bass_kernel_spmd

B = 8

_NC_CACHE = None


def _get_nc():
    global _NC_CACHE
    if _NC_CACHE is None:
        _NC_CACHE = build_nc()
    return _NC_CACHE


def _make_in_maps(inputs):
    x = np.asarray(inputs["x"], np.float16)
    W_qkv = np.asarray(inputs["W_qkv"], np.float64)
    b_qkv = np.asarray(inputs["b_qkv"], np.float64)
    W_proj = np.asarray(inputs["W_proj"], np.float64)
    b_proj = np.asarray(inputs["b_proj"], np.float64)

    s = 1.0 / np.sqrt(np.float64(D))
    wq_s = W_qkv[:, :D] * s
    bq_s = b_qkv[:D] * s
    wk = W_qkv[:, D : 2 * D]
    wv = W_qkv[:, 2 * D :]
    bv = b_qkv[2 * D :]

    shared = {
        # wb[ic, p, oc, col] = B[ic*128+p, oc*128+col]
        "wb": np.ascontiguousarray(
            (wq_s @ wk.T).astype(np.float16).reshape(4, 128, 4, 128)
        ),
        "wvp": np.ascontiguousarray((wv @ W_proj).astype(np.float16)),
        "u": np.ascontiguousarray(
            (wk @ bq_s).astype(np.float16).reshape(4, 128).T
        ),
        "bvp": np.ascontiguousarray(
            (bv @ W_proj + b_proj).astype(np.float32)
        ),
        "ones": np.ones((P, P), np.float16),
    }
    return [{"x": np.ascontiguousarray(x[b]), **shared} for b in range(B)]


def kernel(**inputs):
    nc = _get_nc()
    in_maps = _make_in_maps(inputs)
    res = run_bass_kernel_spmd(nc, in_maps, core_ids=list(range(B)))
    return np.stack([res.results[b]["y"] for b in range(B)]).astype(np.float32)
